# revision 22
# baseline (speedup 1.0000x reference)
"""ARIMA(0,1,1) innovations kernel for 8 TRN2 NeuronCores.

Math: the reference solves the min-norm least-squares problem A x = b where
A is the N x (N+1) bidiagonal MA(1) matrix (c on the diagonal, 1 on the
superdiagonal), b = diff(time_block) - arma_const, and returns x / std.

Every x with A x = b satisfies x_{i+1} = b_i - c*x_i, so the solution set is
x(t) = xhat + t*h with xhat = [0, f] (f the IIR scan f_i = s*f_{i-1} + b_i,
s = -c) and h_i = s^i spanning null(A).  The min-norm solution projects out
h:  x = xhat - rho*h with rho = <xhat,h>/||h||^2 = sum_j b_j s^{j+1} (exact
to f32 for |c| < 1; dropped terms are O(s^N), N = 4096).  The whole problem
is one first-order scan plus one rank-one correction; 1/std is folded into
b so the final scan directly emits the output.

Layout: b is blocked [128, 32] (partition p holds elements 32p..32p+31).
Local scans are single native tensor_tensor_scan instructions.  The
cross-partition carries C[p] = s^32*C[p-1] + f_loc[p,31] are an exact
128-step scan computed in row space: one PE transpose (stationary operand
[zero-col | floc-col31] via a strided AP) feeds a 2-partition scan that
simultaneously produces the carry row and the s^{32p} row; a second
transpose of the per-partition rho partials gives rho by a row dot; one
K=2 matmul then produces the per-partition initial states
C[p-1] - rho*s^{32p}, and the final scan emits the answer.

The production path (build_nc_raw) is raw bass (Block + manual
semaphores): waits are attached to instructions, every DVE op bumps a
self-semaphore (the DVE pipe does not interlock same-engine RAW), PE->DVE
edges drain the PE pipe after transposes, and the three DMAs (time_block
via one overlapping access pattern, pre-replicated scalars, output) are
first-in-queue on separate HWDGE queues.  The transpose identity is built
on gpsimd (memset + affine_select) during the DMA wait.  No collectives:
the problem is 16 KB in/out, so all 8 cores run the identical program
(data-parallel replication per the sharding hint) and the host takes core
0's output.  A TileContext reference implementation (build_nc) is kept
for debugging.

Assumes 0 < |ma_coeff| < 1 (reference setup uses c = 0.5; at |c| -> 1 the
geometric-series identity for the projection coefficient degrades).
"""

import numpy as np

N = 4096
P = 128
Q = 32

_CACHE: dict = {}


def _ensure_paths():
    import sys
    for p in ("/opt/trn_rl_repo", "/root/.axon_site", "/root/.axon_site/_ro/trn_rl_repo",
              "/root/.axon_site/_ro/pypackages"):
        if p not in sys.path:
            sys.path.append(p)


def build_nc():
    """Build the Bass/Tile graph (single SPMD program, run on all 8 cores)."""
    _ensure_paths()
    import concourse.bass as bass
    import concourse.mybir as mybir
    from concourse import bacc, tile

    dt = mybir.dt.float32
    OP = mybir.AluOpType

    nc = bacc.Bacc(None, target_bir_lowering=False)

    tb_d = nc.dram_tensor("time_block", [N + 1], dt, kind="ExternalInput")
    const_d = nc.dram_tensor("arma_const", [1], dt, kind="ExternalInput")
    coeff_d = nc.dram_tensor("ma_coeff", [1], dt, kind="ExternalInput")
    std_d = nc.dram_tensor("std_innovation", [1], dt, kind="ExternalInput")
    ident_d = nc.dram_tensor("ident", [P, P], dt, kind="ExternalInput")
    out_d = nc.dram_tensor("out", [N + 1], dt, kind="ExternalOutput")

    with tile.TileContext(nc) as tc:
        with (
            tc.tile_pool(name="sb", bufs=1) as sb,
            tc.tile_pool(name="ps", bufs=1, space=bass.MemorySpace.PSUM) as ps,
        ):
            TBa = sb.tile([P, Q], dt)
            TBb = sb.tile([P, Q], dt)
            IDT = sb.tile([P, P], dt)
            c1t = sb.tile([1, 1], dt)
            cct = sb.tile([1, 1], dt)
            sdt = sb.tile([1, 1], dt)
            SC = sb.tile([1, 16], dt)      # scalar scratch row (partition 0)
            ONESR = sb.tile([1, P], dt)    # ones row
            ZROW = sb.tile([1, P], dt)     # zero row
            BCT = sb.tile([P, 3], dt)      # [const, s, istd] broadcast
            ZT = sb.tile([P, Q], dt)       # zeros
            ST = sb.tile([P, Q], dt)       # s replicated
            POW = sb.tile([P, Q + 1], dt)  # POW[p, j] = s^j
            S32R = sb.tile([1, P], dt)     # s^32 replicated row
            SPROW = sb.tile([1, P + 1], dt)  # col j = s^{32j}
            B0 = sb.tile([P, Q], dt)
            B = sb.tile([P, Q], dt)
            FLOC = sb.tile([P, Q], dt)
            CR = sb.tile([1, P + 1], dt)   # carry row, col p+1 = C[p]
            Ff = sb.tile([P, Q], dt)
            F = sb.tile([P, Q], dt)
            WP = sb.tile([P, Q], dt)       # B * s^{q+1}
            WC = sb.tile([P, 1], dt)       # row sums of WP
            SPC = sb.tile([P, 1], dt)      # s^{32p} column
            RHOc = sb.tile([1, 1], dt)     # rho
            NEGR = sb.tile([1, 1], dt)     # -rho
            O0 = sb.tile([1, 1], dt)       # -rho*istd

            psBC = ps.tile([P, 3], dt)
            psROW = ps.tile([1, P], dt)
            psC = ps.tile([P, 1], dt)
            psS = ps.tile([P, 1], dt)
            psR = ps.tile([1, 1], dt)

            V = nc.vector
            G = nc.gpsimd
            tt = V.tensor_tensor
            tts = V.tensor_tensor_scan
            gt = G.tensor_tensor

            # ---- input DMAs, spread across the three DMA-capable queues;
            # scalars first (they gate the setup chain) ----
            nc.sync.dma_start(out=cct[:], in_=coeff_d[None, :])
            nc.scalar.dma_start(out=c1t[:], in_=const_d[None, :])
            G.dma_start(out=sdt[:], in_=std_d[None, :])
            nc.sync.dma_start(out=TBa[:], in_=tb_d[0:N].rearrange("(p q) -> p q", p=P))
            nc.scalar.dma_start(out=TBb[:], in_=tb_d[1:N + 1].rearrange("(p q) -> p q", p=P))
            G.dma_start(out=IDT[:], in_=ident_d[:])

            # ---- compile-time constants + gpsimd tensor-op warmup ----
            G.memset(ONESR[:], 1.0)
            G.memset(ZROW[:], 0.0)
            G.memset(ZT[:], 0.0)
            G.memset(CR[0:1, 0:1], 0.0)
            G.memset(SPROW[0:1, 0:1], 1.0)
            G.memset(POW[:, 0:1], 1.0)
            G.tensor_copy(WP[0:1, 0:2], ZROW[0:1, 0:2])  # eat gpsimd ucode load

            # ---- scalar assembly on partition 0 (DVE) ----
            # SC cols: 0=s 1=const 2=istd
            V.tensor_scalar_mul(SC[0:1, 0:1], cct[:], -1.0)
            V.tensor_copy(SC[0:1, 1:2], c1t[:])
            V.reciprocal(SC[0:1, 2:3], sdt[:])

            # broadcast [s, const, istd] to all partitions
            nc.tensor.matmul(psBC[:], ONESR[0:1, 0:P], SC[0:1, 0:3])
            V.tensor_copy(BCT[:], psBC[:])
            sB = BCT[:, 0:1]
            constB = BCT[:, 1:2]
            istdB = BCT[:, 2:3]

            # s tile + power vectors (POW col 32 doubles as s^32)
            V.tensor_scalar_add(ST[:], ZT[:], sB)
            tts(POW[:, 1:Q + 1], ST[:], ZT[:], 1.0, OP.mult, OP.add)
            G.tensor_scalar_mul(S32R[:], ONESR[:], POW[0:1, Q:Q + 1])

            # ---- main chain (DVE + PE) ----
            tt(B0[:], TBb[:], TBa[:], OP.subtract)
            # b' = (diff - const) * istd  — istd folded in, so the final scan
            # directly produces the output
            V.tensor_scalar(B[:], B0[:], constB, istdB, OP.subtract, OP.mult)
            tts(FLOC[:], ST[:], B[:], 0.0, OP.mult, OP.add)
            nc.tensor.transpose(psROW[:], FLOC[:, Q - 1:Q], IDT[:])
            tts(SPROW[0:1, 1:P + 1], S32R[:], ZROW[:], 1.0, OP.mult, OP.add)
            tts(CR[0:1, 1:P + 1], S32R[:], psROW[:], 0.0, OP.mult, OP.add)

            # ---- rho chain (gpsimd + PE), racing the carry chain ----
            gt(WP[:], B[:], POW[:, 1:Q + 1], OP.mult)
            V.tensor_reduce(WC[:], WP[:], mybir.AxisListType.X, OP.add)
            nc.tensor.transpose(psS[:], SPROW[0:1, 0:P], ONESR[0:1, 0:1])
            V.tensor_copy(SPC[:], psS[:, 0:1])
            nc.tensor.matmul(psR[:], WC[:, 0:1], SPC[:, 0:1])
            V.tensor_scalar_mul(NEGR[:], psR[:], -1.0)  # -rho' = x_0

            # carry column + folded correction: psC[p] = C[p-1] - rho'*s^{32p}
            nc.tensor.matmul(psC[:], CR[0:1, 0:P], ONESR[0:1, 0:1],
                             start=True, stop=False)
            nc.tensor.matmul(psC[:], SPROW[0:1, 0:P], NEGR[0:1, 0:1],
                             start=False, stop=True)

            # final scan directly yields the output block x[1:4097]
            tts(Ff[:], ST[:], B[:], psC[:, 0:1], OP.mult, OP.add)

            # ---- stores ----
            nc.sync.dma_start(out=out_d[1:N + 1].rearrange("(p q) -> p q", p=P), in_=Ff[:])
            nc.scalar.dma_start(out=out_d[0:1][None, :], in_=NEGR[:])

    nc.compile()
    return nc


def _get_nc():
    if "nc" not in _CACHE:
        _CACHE["nc"] = build_nc_raw()
    return _CACHE["nc"]


def _in_map(inputs):
    aux = _aux_array()
    aux[:, 0] = np.float32(np.asarray(inputs["ma_coeff"]).reshape(-1)[0])
    aux[:, 1] = np.float32(np.asarray(inputs["arma_const"]).reshape(-1)[0])
    aux[:, 2] = np.float32(np.asarray(inputs["std_innovation"]).reshape(-1)[0])
    return {
        "time_block": np.ascontiguousarray(np.asarray(inputs["time_block"], dtype=np.float32)),
        "aux": aux,
    }


def run(inputs, trace=False, tmpdir=None):
    """Run on all 8 cores (replicated); returns (output, BassKernelResults)."""
    _ensure_paths()
    from concourse.bass_utils import run_bass_kernel_spmd

    nc = _get_nc()
    m = _in_map(inputs)
    res = run_bass_kernel_spmd(nc, [m] * 8, list(range(8)), trace=trace, tmpdir=tmpdir)
    return res.results[0]["out"].reshape(N + 1).astype(np.float32), res


def kernel(**inputs) -> np.ndarray:
    out, _ = run(inputs)
    return out

def build_nc_raw():
    """Raw-bass (Block + manual semaphores).

    Same-engine RAW/WAW on DVE requires a self-semaphore wait (the pipe
    does not interlock); every DVE op incs `vs` and dependents wait on it.
    PE -> DVE edges go through a PE drain so PSUM writeback is complete.
    DVE ops must start at partition 0, so the carry row and the s^{32p}
    row are computed together by one 2-partition scan fed from a single
    PE transpose whose stationary operand is [zero-col | floc-col31]
    (strided free AP), and the correction is one K=2 matmul.  The three
    scalar params arrive pre-replicated across partitions (aux [128,3]);
    the transpose identity is built on gpsimd (memset + affine_select)
    during the DMA window.
    """
    _ensure_paths()
    from contextlib import ExitStack
    import concourse.bass as bass
    import concourse.mybir as mybir

    dt = mybir.dt.float32
    OP = mybir.AluOpType

    nc = bass.Bass()

    tb_d = nc.dram_tensor("time_block", [N + 1], dt, kind="ExternalInput")
    aux_d = nc.dram_tensor("aux", [P, 3], dt, kind="ExternalInput")
    out_d = nc.dram_tensor("out", [N + 1], dt, kind="ExternalOutput")

    ctx = ExitStack()
    t = lambda name, shape: ctx.enter_context(nc.sbuf_tensor(name, shape, dt))
    pt = lambda name, shape: ctx.enter_context(nc.psum_tensor(name, shape, dt))
    with ctx:
        TB33 = t("TB33", [P, Q + 1])  # TB33[p, j] = tb[32p + j]
        BCT = t("BCT", [P, 3])       # replicated [c, const, std]
        IDT = t("IDT", [P, P])       # identity, built on gpsimd
        ZCOL = t("ZCOL", [P, 1])
        SNEG = t("SNEG", [P, 1])     # s = -c
        ISTD = t("ISTD", [P, 1])
        POW32 = t("POW32", [P, Q])   # col q = s^{q+1}
        FLOC = t("FLOC", [P, Q + 1])  # col0 = 0, cols 1..32 = local scan
        CRSP = t("CRSP", [2, P + 1])  # row0: s^{32j}; row1: carry row
        INIT2 = t("INIT2", [2, 1])   # [1.0; 0.0]
        B0 = t("B0", [P, Q])
        B = t("B", [P, Q])
        WP = t("WP", [P, Q])
        WC = t("WC", [P, 1])
        RB = t("RB", [1, P])
        RH = t("RH", [1, 1])
        RHS2 = t("RHS2", [2, 1])     # [ -rho ; 1.0 ]
        Ff = t("Ff", [P, Q])

        psROW2 = pt("psROW2", [2, P])
        psW = pt("psW", [1, P])
        psC = pt("psC", [P, 1])

        dS = ctx.enter_context(nc.semaphore("dS"))
        dA = ctx.enter_context(nc.semaphore("dA"))
        vs = ctx.enter_context(nc.semaphore("vs"))
        pp = ctx.enter_context(nc.semaphore("pp"))
        gs = ctx.enter_context(nc.semaphore("gs"))

        blk = ctx.enter_context(nc.Block())

        import bass_rust as _br
        tb_overlap = _br.AP(tb_d[0:1].tensor, 0, [[Q, P], [1, Q + 1]])

        @blk.sync
        def _(sync):
            sync.dma_start(out=TB33[:], in_=tb_overlap).then_inc(dS, 16)
            sync.dma_start(
                out=out_d[1:N // 2 + 1].rearrange("(p q) -> p q", p=P // 2),
                in_=Ff[0:P // 2, :]
            )._wait_ge(vs, 19).then_inc(dS, 16)

        @blk.scalar
        def _(scalar):
            scalar.dma_start(out=BCT[:], in_=aux_d[:]).then_inc(dA, 16)
            scalar.dma_start(
                out=out_d[0:1][None, :], in_=RHS2[0:1, 0:1]
            )._wait_ge(vs, 18).then_inc(dA, 16)
            scalar.dma_start(
                out=out_d[N // 2 + 1:N + 1].rearrange("(p q) -> p q", p=P // 2),
                in_=Ff[P // 2:P, :]
            )._wait_ge(vs, 20).then_inc(dA, 16)

        @blk.gpsimd
        def _(gpsimd):
            G = nc.gpsimd
            G.memset(IDT[:], 1.0)
            G.affine_select(IDT[:], IDT[:], pattern=[[1, P]],
                            compare_op=mybir.AluOpType.is_equal, fill=0.0,
                            base=0, channel_multiplier=-1).then_inc(gs, 1)

        @blk.vector
        def _(vector):
            V = nc.vector
            tt = V.tensor_tensor
            tts = V.tensor_tensor_scan
            V.memset(ZCOL[:], 0.0).then_inc(vs, 1)              # 1
            V.memset(CRSP[0:2, 0:1], 0.0).then_inc(vs, 1)       # 2
            V.memset(CRSP[0:1, 0:1], 1.0)._wait_ge(vs, 2).then_inc(vs, 1)  # 3
            V.memset(RHS2[0:2, 0:1], 1.0).then_inc(vs, 1)       # 4
            V.memset(INIT2[0:2, 0:1], 0.0).then_inc(vs, 1)      # 5
            V.memset(INIT2[0:1, 0:1], 1.0)._wait_ge(vs, 5).then_inc(vs, 1)  # 6
            V.memset(FLOC[:, 0:1], 0.0).then_inc(vs, 1)         # 7
            tt(B0[:], TB33[:, 1:Q + 1], TB33[:, 0:Q],
               OP.subtract)._wait_ge(dS, 16).then_inc(vs, 1)                # 8
            V.reciprocal(ISTD[:], BCT[:, 2:3])._wait_ge(dA, 16).then_inc(vs, 1)  # 9
            V.tensor_scalar_mul(SNEG[:], BCT[:, 0:1], -1.0).then_inc(vs, 1)  # 10
            V.tensor_scalar(B[:], B0[:], BCT[:, 1:2], ISTD[:, 0:1],
                            OP.subtract, OP.mult)._wait_ge(vs, 9).then_inc(vs, 1)  # 11
            tts(FLOC[:, 1:Q + 1], SNEG[:, 0:1].broadcast_to((P, Q)), B[:],
                0.0, OP.mult, OP.add)._wait_ge(vs, 11).then_inc(vs, 1)      # 12
            tts(POW32[:], SNEG[:, 0:1].broadcast_to((P, Q)),
                ZCOL[:, 0:1].broadcast_to((P, Q)), 1.0, OP.mult,
                OP.add).then_inc(vs, 1)                                     # 13
            tt(WP[:], B[:], POW32[:], OP.mult)._wait_ge(vs, 13).then_inc(vs, 1)  # 14
            V.tensor_reduce(WC[:], WP[:], mybir.AxisListType.X,
                            OP.add)._wait_ge(vs, 14).then_inc(vs, 1)        # 15
            tts(CRSP[0:2, 1:P + 1],
                POW32[0:2, Q - 1:Q].broadcast_to((2, P)),
                psROW2[0:2, 0:P], INIT2[0:2, 0:1], OP.mult,
                OP.add)._wait_ge(pp, 1).then_inc(vs, 1)                     # 16
            tt(RB[:], psW[0:1, 0:P], CRSP[0:1, 0:P],
               OP.mult)._wait_ge(vs, 16).then_inc(vs, 1)                    # 17
            V.tensor_reduce(RHS2[0:1, 0:1], RB[:], mybir.AxisListType.X,
                            OP.add, negate=True)._wait_ge(vs, 17).then_inc(vs, 1)  # 18
            H = P // 2
            tts(Ff[0:H, :], SNEG[0:H, 0:1].broadcast_to((H, Q)), B[0:H, :],
                psC[0:H, 0:1], OP.mult, OP.add)._wait_ge(pp, 2).then_inc(vs, 1)  # 19
            tts(Ff[H:P, :], SNEG[H:P, 0:1].broadcast_to((H, Q)), B[H:P, :],
                psC[H:P, 0:1], OP.mult, OP.add).then_inc(vs, 1)             # 20

        @blk.tensor
        def _(tensor):
            T = nc.tensor
            tensor.wait_ge(gs, 1)
            # stationary = [zero-col | floc-col31] via strided free AP
            T.transpose(psROW2[:], FLOC[:, 0:Q + 1:Q], IDT[:])._wait_ge(vs, 12)
            T.transpose(psW[:], WC[:, 0:1], IDT[:])._wait_ge(vs, 15)
            T.drain().then_inc(pp, 1)                           # p=1
            T.matmul(psC[:], CRSP[0:2, 0:P],
                     RHS2[0:2, 0:1])._wait_ge(vs, 18).then_inc(pp, 1)  # p=2

    return nc


def _aux_array():
    return np.zeros((P, 3), dtype=np.float32)


# revision 23
# speedup vs baseline: 1.1474x; 1.1474x over previous
"""ARIMA(0,1,1) innovations kernel for 8 TRN2 NeuronCores.

Math: the reference solves the min-norm least-squares problem A x = b where
A is the N x (N+1) bidiagonal MA(1) matrix (c on the diagonal, 1 on the
superdiagonal), b = diff(time_block) - arma_const, and returns x / std.

Every x with A x = b satisfies x_{i+1} = b_i - c*x_i, so the solution set is
x(t) = xhat + t*h with xhat = [0, f] (f the IIR scan f_i = s*f_{i-1} + b_i,
s = -c) and h_i = s^i spanning null(A).  The min-norm solution projects out
h:  x = xhat - rho*h with rho = <xhat,h>/||h||^2 = sum_j b_j s^{j+1} (exact
to f32 for |c| < 1; dropped terms are O(s^N), N = 4096).  The whole problem
is one first-order scan plus one rank-one correction; 1/std is folded into
b so the final scan directly emits the output.

Layout: b is blocked [128, 32] (partition p holds elements 32p..32p+31).
Local scans are single native tensor_tensor_scan instructions.  The
cross-partition carries C[p] = s^32*C[p-1] + f_loc[p,31] are an exact
128-step scan computed in row space: one PE transpose (stationary operand
[zero-col | floc-col31] via a strided AP) feeds a 2-partition scan that
simultaneously produces the carry row and the s^{32p} row; a second
transpose of the per-partition rho partials gives rho by a row dot; one
K=2 matmul then produces the per-partition initial states
C[p-1] - rho*s^{32p}, and the final scan emits the answer.

The production path (build_nc_raw) is raw bass (Block + manual
semaphores): waits are attached to instructions, every DVE op bumps a
self-semaphore (the DVE pipe does not interlock same-engine RAW), PE->DVE
edges drain the PE pipe after transposes, and the three DMAs (time_block
via one overlapping access pattern, pre-replicated scalars, output) are
first-in-queue on separate HWDGE queues.  The transpose identity is built
on gpsimd (memset + affine_select) during the DMA wait.  No collectives:
the problem is 16 KB in/out, so all 8 cores run the identical program
(data-parallel replication per the sharding hint) and the host takes core
0's output.  A TileContext reference implementation (build_nc) is kept
for debugging.

Assumes 0 < |ma_coeff| < 1 (reference setup uses c = 0.5; at |c| -> 1 the
geometric-series identity for the projection coefficient degrades).
"""

import numpy as np

N = 4096
P = 128
Q = 32

_CACHE: dict = {}


def _ensure_paths():
    import sys
    for p in ("/opt/trn_rl_repo", "/root/.axon_site", "/root/.axon_site/_ro/trn_rl_repo",
              "/root/.axon_site/_ro/pypackages"):
        if p not in sys.path:
            sys.path.append(p)


def build_nc():
    """Build the Bass/Tile graph (single SPMD program, run on all 8 cores)."""
    _ensure_paths()
    import concourse.bass as bass
    import concourse.mybir as mybir
    from concourse import bacc, tile

    dt = mybir.dt.float32
    OP = mybir.AluOpType

    nc = bacc.Bacc(None, target_bir_lowering=False)

    tb_d = nc.dram_tensor("time_block", [N + 1], dt, kind="ExternalInput")
    const_d = nc.dram_tensor("arma_const", [1], dt, kind="ExternalInput")
    coeff_d = nc.dram_tensor("ma_coeff", [1], dt, kind="ExternalInput")
    std_d = nc.dram_tensor("std_innovation", [1], dt, kind="ExternalInput")
    ident_d = nc.dram_tensor("ident", [P, P], dt, kind="ExternalInput")
    out_d = nc.dram_tensor("out", [N + 1], dt, kind="ExternalOutput")

    with tile.TileContext(nc) as tc:
        with (
            tc.tile_pool(name="sb", bufs=1) as sb,
            tc.tile_pool(name="ps", bufs=1, space=bass.MemorySpace.PSUM) as ps,
        ):
            TBa = sb.tile([P, Q], dt)
            TBb = sb.tile([P, Q], dt)
            IDT = sb.tile([P, P], dt)
            c1t = sb.tile([1, 1], dt)
            cct = sb.tile([1, 1], dt)
            sdt = sb.tile([1, 1], dt)
            SC = sb.tile([1, 16], dt)      # scalar scratch row (partition 0)
            ONESR = sb.tile([1, P], dt)    # ones row
            ZROW = sb.tile([1, P], dt)     # zero row
            BCT = sb.tile([P, 3], dt)      # [const, s, istd] broadcast
            ZT = sb.tile([P, Q], dt)       # zeros
            ST = sb.tile([P, Q], dt)       # s replicated
            POW = sb.tile([P, Q + 1], dt)  # POW[p, j] = s^j
            S32R = sb.tile([1, P], dt)     # s^32 replicated row
            SPROW = sb.tile([1, P + 1], dt)  # col j = s^{32j}
            B0 = sb.tile([P, Q], dt)
            B = sb.tile([P, Q], dt)
            FLOC = sb.tile([P, Q], dt)
            CR = sb.tile([1, P + 1], dt)   # carry row, col p+1 = C[p]
            Ff = sb.tile([P, Q], dt)
            F = sb.tile([P, Q], dt)
            WP = sb.tile([P, Q], dt)       # B * s^{q+1}
            WC = sb.tile([P, 1], dt)       # row sums of WP
            SPC = sb.tile([P, 1], dt)      # s^{32p} column
            RHOc = sb.tile([1, 1], dt)     # rho
            NEGR = sb.tile([1, 1], dt)     # -rho
            O0 = sb.tile([1, 1], dt)       # -rho*istd

            psBC = ps.tile([P, 3], dt)
            psROW = ps.tile([1, P], dt)
            psC = ps.tile([P, 1], dt)
            psS = ps.tile([P, 1], dt)
            psR = ps.tile([1, 1], dt)

            V = nc.vector
            G = nc.gpsimd
            tt = V.tensor_tensor
            tts = V.tensor_tensor_scan
            gt = G.tensor_tensor

            # ---- input DMAs, spread across the three DMA-capable queues;
            # scalars first (they gate the setup chain) ----
            nc.sync.dma_start(out=cct[:], in_=coeff_d[None, :])
            nc.scalar.dma_start(out=c1t[:], in_=const_d[None, :])
            G.dma_start(out=sdt[:], in_=std_d[None, :])
            nc.sync.dma_start(out=TBa[:], in_=tb_d[0:N].rearrange("(p q) -> p q", p=P))
            nc.scalar.dma_start(out=TBb[:], in_=tb_d[1:N + 1].rearrange("(p q) -> p q", p=P))
            G.dma_start(out=IDT[:], in_=ident_d[:])

            # ---- compile-time constants + gpsimd tensor-op warmup ----
            G.memset(ONESR[:], 1.0)
            G.memset(ZROW[:], 0.0)
            G.memset(ZT[:], 0.0)
            G.memset(CR[0:1, 0:1], 0.0)
            G.memset(SPROW[0:1, 0:1], 1.0)
            G.memset(POW[:, 0:1], 1.0)
            G.tensor_copy(WP[0:1, 0:2], ZROW[0:1, 0:2])  # eat gpsimd ucode load

            # ---- scalar assembly on partition 0 (DVE) ----
            # SC cols: 0=s 1=const 2=istd
            V.tensor_scalar_mul(SC[0:1, 0:1], cct[:], -1.0)
            V.tensor_copy(SC[0:1, 1:2], c1t[:])
            V.reciprocal(SC[0:1, 2:3], sdt[:])

            # broadcast [s, const, istd] to all partitions
            nc.tensor.matmul(psBC[:], ONESR[0:1, 0:P], SC[0:1, 0:3])
            V.tensor_copy(BCT[:], psBC[:])
            sB = BCT[:, 0:1]
            constB = BCT[:, 1:2]
            istdB = BCT[:, 2:3]

            # s tile + power vectors (POW col 32 doubles as s^32)
            V.tensor_scalar_add(ST[:], ZT[:], sB)
            tts(POW[:, 1:Q + 1], ST[:], ZT[:], 1.0, OP.mult, OP.add)
            G.tensor_scalar_mul(S32R[:], ONESR[:], POW[0:1, Q:Q + 1])

            # ---- main chain (DVE + PE) ----
            tt(B0[:], TBb[:], TBa[:], OP.subtract)
            # b' = (diff - const) * istd  — istd folded in, so the final scan
            # directly produces the output
            V.tensor_scalar(B[:], B0[:], constB, istdB, OP.subtract, OP.mult)
            tts(FLOC[:], ST[:], B[:], 0.0, OP.mult, OP.add)
            nc.tensor.transpose(psROW[:], FLOC[:, Q - 1:Q], IDT[:])
            tts(SPROW[0:1, 1:P + 1], S32R[:], ZROW[:], 1.0, OP.mult, OP.add)
            tts(CR[0:1, 1:P + 1], S32R[:], psROW[:], 0.0, OP.mult, OP.add)

            # ---- rho chain (gpsimd + PE), racing the carry chain ----
            gt(WP[:], B[:], POW[:, 1:Q + 1], OP.mult)
            V.tensor_reduce(WC[:], WP[:], mybir.AxisListType.X, OP.add)
            nc.tensor.transpose(psS[:], SPROW[0:1, 0:P], ONESR[0:1, 0:1])
            V.tensor_copy(SPC[:], psS[:, 0:1])
            nc.tensor.matmul(psR[:], WC[:, 0:1], SPC[:, 0:1])
            V.tensor_scalar_mul(NEGR[:], psR[:], -1.0)  # -rho' = x_0

            # carry column + folded correction: psC[p] = C[p-1] - rho'*s^{32p}
            nc.tensor.matmul(psC[:], CR[0:1, 0:P], ONESR[0:1, 0:1],
                             start=True, stop=False)
            nc.tensor.matmul(psC[:], SPROW[0:1, 0:P], NEGR[0:1, 0:1],
                             start=False, stop=True)

            # final scan directly yields the output block x[1:4097]
            tts(Ff[:], ST[:], B[:], psC[:, 0:1], OP.mult, OP.add)

            # ---- stores ----
            nc.sync.dma_start(out=out_d[1:N + 1].rearrange("(p q) -> p q", p=P), in_=Ff[:])
            nc.scalar.dma_start(out=out_d[0:1][None, :], in_=NEGR[:])

    nc.compile()
    return nc


def _get_nc():
    if "nc" not in _CACHE:
        _CACHE["nc"] = build_nc_raw()
    return _CACHE["nc"]


def _in_map(inputs):
    aux = _aux_array()
    aux[:, 0] = np.float32(np.asarray(inputs["ma_coeff"]).reshape(-1)[0])
    aux[:, 1] = np.float32(np.asarray(inputs["arma_const"]).reshape(-1)[0])
    aux[:, 2] = np.float32(np.asarray(inputs["std_innovation"]).reshape(-1)[0])
    return {
        "time_block": np.ascontiguousarray(np.asarray(inputs["time_block"], dtype=np.float32)),
        "aux": aux,
    }


def run(inputs, trace=False, tmpdir=None):
    """Run on all 8 cores (replicated); returns (output, BassKernelResults)."""
    _ensure_paths()
    from concourse.bass_utils import run_bass_kernel_spmd

    nc = _get_nc()
    m = _in_map(inputs)
    res = run_bass_kernel_spmd(nc, [m] * 8, list(range(8)), trace=trace, tmpdir=tmpdir)
    return res.results[0]["out"].reshape(N + 1).astype(np.float32), res


def kernel(**inputs) -> np.ndarray:
    out, _ = run(inputs)
    return out

def build_nc_raw():
    """Raw-bass (Block + manual semaphores).

    Same-engine RAW/WAW on DVE requires a self-semaphore wait (the pipe
    does not interlock); every DVE op incs `vs` and dependents wait on it.
    PE -> DVE edges go through a PE drain so PSUM writeback is complete.
    DVE ops must start at partition 0, so the carry row and the s^{32p}
    row are computed together by one 2-partition scan fed from a single
    PE transpose whose stationary operand is [zero-col | floc-col31]
    (strided free AP), and the correction is one K=2 matmul.  The three
    scalar params arrive pre-replicated across partitions (aux [128,3]);
    the transpose identity is built on gpsimd (memset + affine_select)
    during the DMA window.
    """
    _ensure_paths()
    from contextlib import ExitStack
    import concourse.bass as bass
    import concourse.mybir as mybir

    dt = mybir.dt.float32
    OP = mybir.AluOpType

    nc = bass.Bass()

    tb_d = nc.dram_tensor("time_block", [N + 1], dt, kind="ExternalInput")
    aux_d = nc.dram_tensor("aux", [P, 3], dt, kind="ExternalInput")
    out_d = nc.dram_tensor("out", [N + 1], dt, kind="ExternalOutput")

    ctx = ExitStack()
    t = lambda name, shape: ctx.enter_context(nc.sbuf_tensor(name, shape, dt))
    pt = lambda name, shape: ctx.enter_context(nc.psum_tensor(name, shape, dt))
    with ctx:
        TB33 = t("TB33", [P, Q + 1])  # TB33[p, j] = tb[32p + j]
        BCT = t("BCT", [P, 3])       # replicated [c, const, std]
        IDT = t("IDT", [P, P])       # identity, built on gpsimd
        ZCOL = t("ZCOL", [P, 1])
        SNEG = t("SNEG", [P, 1])     # s = -c
        ISTD = t("ISTD", [P, 1])
        POW32 = t("POW32", [P, Q])   # col q = s^{q+1}
        FLOC = t("FLOC", [P, Q + 1])  # col0 = 0, cols 1..32 = local scan
        CRSP = t("CRSP", [2, P + 1])  # row0: s^{32j}; row1: carry row
        INIT2 = t("INIT2", [2, 1])   # [1.0; 0.0]
        B0 = t("B0", [P, Q])
        B = t("B", [P, Q])
        WP = t("WP", [P, Q])
        WC = t("WC", [P, 1])
        RB = t("RB", [1, P])
        RH = t("RH", [1, 1])
        RHS2 = t("RHS2", [2, 1])     # [ -rho ; 1.0 ]
        Ff = t("Ff", [P, Q])

        psROW2 = pt("psROW2", [2, P])
        psW = pt("psW", [1, P])
        psC = pt("psC", [P, 1])

        dS = ctx.enter_context(nc.semaphore("dS"))
        dA = ctx.enter_context(nc.semaphore("dA"))
        vs = ctx.enter_context(nc.semaphore("vs"))
        pp = ctx.enter_context(nc.semaphore("pp"))
        gs = ctx.enter_context(nc.semaphore("gs"))

        blk = ctx.enter_context(nc.Block())

        import bass_rust as _br
        tb_overlap = _br.AP(tb_d[0:1].tensor, 0, [[Q, P], [1, Q + 1]])

        @blk.sync
        def _(sync):
            sync.dma_start(out=TB33[:], in_=tb_overlap).then_inc(dS, 16)
            sync.dma_start(
                out=out_d[1:N // 2 + 1].rearrange("(p q) -> p q", p=P // 2),
                in_=Ff[0:P // 2, :]
            )._wait_ge(vs, 19).then_inc(dS, 16)

        @blk.scalar
        def _(scalar):
            scalar.dma_start(out=BCT[:], in_=aux_d[:]).then_inc(dA, 16)
            scalar.dma_start(
                out=out_d[0:1][None, :], in_=RHS2[0:1, 0:1]
            )._wait_ge(vs, 18).then_inc(dA, 16)
            scalar.dma_start(
                out=out_d[N // 2 + 1:N + 1].rearrange("(p q) -> p q", p=P // 2),
                in_=Ff[P // 2:P, :]
            )._wait_ge(vs, 20).then_inc(dA, 16)

        @blk.gpsimd
        def _(gpsimd):
            G = nc.gpsimd
            G.memset(IDT[:], 1.0)
            G.affine_select(IDT[:], IDT[:], pattern=[[1, P]],
                            compare_op=mybir.AluOpType.is_equal, fill=0.0,
                            base=0, channel_multiplier=-1).then_inc(gs, 1)

        @blk.vector
        def _(vector):
            V = nc.vector
            tt = V.tensor_tensor
            tts = V.tensor_tensor_scan
            V.memset(ZCOL[:], 0.0).then_inc(vs, 1)              # 1
            V.memset(CRSP[0:2, 0:1], 0.0).then_inc(vs, 1)       # 2
            V.memset(CRSP[0:1, 0:1], 1.0)._wait_ge(vs, 2).then_inc(vs, 1)  # 3
            V.memset(RHS2[0:2, 0:1], 1.0).then_inc(vs, 1)       # 4
            V.memset(INIT2[0:2, 0:1], 0.0).then_inc(vs, 1)      # 5
            V.memset(INIT2[0:1, 0:1], 1.0)._wait_ge(vs, 5).then_inc(vs, 1)  # 6
            V.memset(FLOC[:, 0:1], 0.0).then_inc(vs, 1)         # 7
            V.reciprocal(ISTD[:], BCT[:, 2:3])._wait_ge(dA, 16).then_inc(vs, 1)  # 8
            V.tensor_scalar_mul(SNEG[:], BCT[:, 0:1], -1.0).then_inc(vs, 1)  # 9
            tt(B0[:], TB33[:, 1:Q + 1], TB33[:, 0:Q],
               OP.subtract)._wait_ge(dS, 16).then_inc(vs, 1)                # 10
            V.tensor_scalar(B[:], B0[:], BCT[:, 1:2], ISTD[:, 0:1],
                            OP.subtract, OP.mult)._wait_ge(vs, 10).then_inc(vs, 1)  # 11
            tts(FLOC[:, 1:Q + 1], SNEG[:, 0:1].broadcast_to((P, Q)), B[:],
                0.0, OP.mult, OP.add)._wait_ge(vs, 11).then_inc(vs, 1)      # 12
            tts(POW32[:], SNEG[:, 0:1].broadcast_to((P, Q)),
                ZCOL[:, 0:1].broadcast_to((P, Q)), 1.0, OP.mult,
                OP.add).then_inc(vs, 1)                                     # 13
            tt(WP[:], B[:], POW32[:], OP.mult)._wait_ge(vs, 13).then_inc(vs, 1)  # 14
            V.tensor_reduce(WC[:], WP[:], mybir.AxisListType.X,
                            OP.add)._wait_ge(vs, 14).then_inc(vs, 1)        # 15
            tts(CRSP[0:2, 1:P + 1],
                POW32[0:2, Q - 1:Q].broadcast_to((2, P)),
                psROW2[0:2, 0:P], INIT2[0:2, 0:1], OP.mult,
                OP.add)._wait_ge(pp, 1).then_inc(vs, 1)                     # 16
            vector.wait_ge(pp, 2)
            tt(RB[:], psW[0:1, 0:P], CRSP[0:1, 0:P],
               OP.mult)._wait_ge(vs, 16).then_inc(vs, 1)                    # 17
            V.tensor_reduce(RHS2[0:1, 0:1], RB[:], mybir.AxisListType.X,
                            OP.add, negate=True)._wait_ge(vs, 17).then_inc(vs, 1)  # 18
            H = P // 2
            tts(Ff[0:H, :], SNEG[0:H, 0:1].broadcast_to((H, Q)), B[0:H, :],
                psC[0:H, 0:1], OP.mult, OP.add)._wait_ge(pp, 3).then_inc(vs, 1)  # 19
            tts(Ff[H:P, :], SNEG[H:P, 0:1].broadcast_to((H, Q)), B[H:P, :],
                psC[H:P, 0:1], OP.mult, OP.add).then_inc(vs, 1)             # 20

        @blk.tensor
        def _(tensor):
            T = nc.tensor
            tensor.wait_ge(gs, 1)
            # stationary = [zero-col | floc-col31] via strided free AP
            T.transpose(psROW2[:], FLOC[:, 0:Q + 1:Q], IDT[:])._wait_ge(vs, 12)
            T.drain().then_inc(pp, 1)                           # p=1
            T.transpose(psW[:], WC[:, 0:1], IDT[:])._wait_ge(vs, 15)
            T.drain().then_inc(pp, 1)                           # p=2
            T.matmul(psC[:], CRSP[0:2, 0:P],
                     RHS2[0:2, 0:1])._wait_ge(vs, 18).then_inc(pp, 1)  # p=3

    return nc


def _aux_array():
    return np.zeros((P, 3), dtype=np.float32)
